# revision 8
# baseline (speedup 1.0000x reference)
"""5G Polar encoder (CRC11 + subchannel alloc + butterfly + interleave) on 8 trn2 cores.

The whole reference computation is GF(2)-linear in u:
    parity  = (u @ crc_gen) mod 2                       -> linear
    bits    = [u | parity] = u @ [I | crc_gen]          -> linear
    scatter x[:, info_pos] = bits                       -> column selection (linear)
    butterfly stages x ^= x[:, g[s]]                    -> linear over GF(2)
    out     = x[:, perm_out]                            -> column gather (linear)

So on the host we compose one binary matrix M [512, 1024] from the tiny index
tables (cheap uint8 ops), and the device kernel is a single fused
    y = (u @ M) mod 2
data-parallel over the batch: each of the 8 cores computes an [8192, 512] @
[512, 1024] matmul in fp8e4 with DoubleRow perf mode (exact: all values are
0/1, sums <= 512 accumulate in f32 PSUM). The mod-2 runs on the eviction
path: ACT converts PSUM f32 -> i16, DVE ANDs with 1, GPSIMD narrows to i8
for the output DMA (host converts {0,1} i8 -> f32).
"""

import numpy as np
import ml_dtypes

N_CORES = 8
BS = 65536
K = 512          # u feature dim (contraction)
N = 1024         # output columns
SHARD = BS // N_CORES  # 8192 batch rows per core
P = 128
KT = K // P      # 4 k-tiles
NB = SHARD // P  # 64 batch tiles per core

FP8_NP = ml_dtypes.float8_e4m3

_nc_cache = {}


def build_M(crc_gen, info_pos, ind_gather, perm_out):
    """Compose the encoder into one GF(2) matrix M [K, N]: out = (u @ M) mod 2."""
    crc_gen = np.asarray(crc_gen)
    info_pos = np.asarray(info_pos)
    ind_gather = np.asarray(ind_gather)
    perm_out = np.asarray(perm_out)
    k, _ = crc_gen.shape
    nb, n1 = ind_gather.shape
    kp = info_pos.shape[0]
    C = (crc_gen.astype(np.int64) & 1).astype(np.uint8)
    B = np.concatenate([np.eye(k, dtype=np.uint8), C], axis=1)  # [k, kp]
    # scatter bits into columns; duplicate indices: last write wins (matches
    # jax/numpy .at[].set application order)
    col_src = np.full(n1, -1, np.int64)
    col_src[info_pos] = np.arange(kp)
    A = np.zeros((k, n1), np.uint8)
    valid = col_src >= 0
    A[:, valid] = B[:, col_src[valid]]
    for s in range(nb):
        A = A ^ A[:, ind_gather[s]]
    return A[:, perm_out]  # [k, n]


def _build_nc(reps=1):
    import concourse.tile as tile
    from concourse import bacc, mybir

    nc = bacc.Bacc("TRN2", target_bir_lowering=False, debug=False)
    fp8 = mybir.dt.float8e4
    f32 = mybir.dt.float32
    i16 = mybir.dt.int16
    i8 = mybir.dt.int8
    DR = mybir.MatmulPerfMode.DoubleRow

    # k-major 3D layouts: [p, ks, free] with global k = ks*128 + p (both
    # operands use the same mapping, so the contraction is correct).
    uT = nc.declare_dram_parameter("uT", [P, KT, SHARD], fp8, isOutput=False)
    mat = nc.declare_dram_parameter("mat", [P, KT, N], fp8, isOutput=False)
    y = nc.declare_dram_parameter("y", [SHARD, N], i8, isOutput=True)

    with tile.TileContext(nc) as tc:
        with (
            tc.tile_pool(name="consts", bufs=1) as cpool,
            tc.tile_pool(name="work", bufs=4) as wpool,
            tc.tile_pool(name="outs", bufs=4) as opool,
            tc.tile_pool(name="psum", bufs=4, space="PSUM") as ppool,
        ):
            mt = cpool.tile([P, KT, N], fp8, tag="mt")
            nc.sync.dma_start(mt[:], mat[:])
            ut = cpool.tile([P, KT, SHARD], fp8, tag="ut")
            nc.sync.dma_start(ut[:], uT[:])
            for b in [b for _ in range(reps) for b in range(NB)]:
                ps = ppool.tile([P, N], f32)
                t16 = wpool.tile([P, N], i16, tag="t16")
                a16 = wpool.tile([P, N], i16, tag="a16")
                ot = opool.tile([P, N], i8, tag="ot")
                for h in range(2):
                    for ks in range(0, KT, 2):
                        nc.tensor.matmul(
                            ps[:, h * 512:(h + 1) * 512],
                            ut[:, ks:ks + 2, b * P:(b + 1) * P],
                            mt[:, ks:ks + 2, h * 512:(h + 1) * 512],
                            start=(ks == 0),
                            stop=(ks == KT - 2),
                            perf_mode=DR,
                        )
                nc.scalar.activation(
                    t16[:], ps[:], mybir.ActivationFunctionType.Copy
                )
                nc.vector.tensor_scalar(
                    a16[:], t16[:], 1, None, mybir.AluOpType.bitwise_and
                )
                nc.gpsimd.tensor_copy(ot[:], a16[:])
                nc.sync.dma_start(y[b * P:(b + 1) * P, :], ot[:])
    nc.compile()
    return nc


def get_nc(reps=1):
    if reps not in _nc_cache:
        _nc_cache[reps] = _build_nc(reps)
    return _nc_cache[reps]


def _to_k_major(a_km, free):
    """[K, free] -> [P, KT, free] with k = ks*128 + p."""
    return np.ascontiguousarray(
        a_km.reshape(KT, P, free).transpose(1, 0, 2)
    )


def make_in_maps(u, M):
    u8 = np.asarray(u).astype(FP8_NP)
    m8 = np.asarray(M).astype(FP8_NP)
    mat3 = _to_k_major(m8, N)
    in_maps = []
    for i in range(N_CORES):
        uT_i = np.ascontiguousarray(u8[i * SHARD:(i + 1) * SHARD, :].T)
        in_maps.append({"uT": _to_k_major(uT_i, SHARD), "mat": mat3})
    return in_maps


def kernel(u, crc_gen, info_pos, ind_gather, perm_out):
    from concourse.bass_utils import run_bass_kernel_spmd

    M = build_M(crc_gen, info_pos, ind_gather, perm_out)
    in_maps = make_in_maps(u, M)
    nc = get_nc()
    res = run_bass_kernel_spmd(nc, in_maps, core_ids=list(range(N_CORES)))
    out = np.concatenate(
        [np.asarray(r["y"]).astype(np.float32) for r in res.results], axis=0
    )
    return out
